# revision 16
# baseline (speedup 1.0000x reference)
"""FBPINN (16 subdomain MLPs over [0,1]^2, cosine partition-of-unity windows)
as a Trainium2 Bass kernel, expert-parallel over 8 NeuronCores.

Sharding: subdomain axis K=16 split 2-per-core. Each core runs its two
subnets on all N points and emits partial (numerator, denominator) sums of
the window-weighted combine; the host adds partials across cores and divides.
This is exact because the window normalization is a ratio of K-sums.

Host-side preprocessing folds the per-subdomain input normalization
xn = (x - center)/scale into the layer-0 weights/bias, so the device
only computes tanh-MLP layers + windows. Matmuls run in float32r
(single-pass PE mode, ~4x the fp32 rate; ~1e-3 relative rounding).
"""

import contextlib
import math

import numpy as np

import concourse.bacc as bacc
import concourse.mybir as mybir
import concourse.tile as tile
from concourse.bass_utils import run_bass_kernel_spmd

# problem constants (hardcoded per harness contract)
K, D, N, W, OUT_DIM = 16, 2, 16384, 256, 1
TW = 0.2
NCORES = 8
KPC = K // NCORES  # subdomains per core
P = 128
NT = N // P        # 128 columns in point-partition layout
CH = 1024          # point chunk through the MLP pipeline
NCH = N // CH
SUB = 512          # matmul moving-operand subchunk
FT = W // P        # feature tiles per hidden layer (2)

F32 = mybir.dt.float32
F32R = mybir.dt.float32r
AF = mybir.ActivationFunctionType
ALU = mybir.AluOpType


def _build_program(repeat=1):
    nc = bacc.Bacc("TRN2", target_bir_lowering=False, debug=False, num_devices=NCORES)

    xT = nc.dram_tensor("XT", [D, N], F32R, kind="ExternalInput")
    xPT = nc.dram_tensor("XPT", [P, D * NT], F32, kind="ExternalInput")
    w0d = nc.dram_tensor("W0S", [D, KPC * W], F32R, kind="ExternalInput")
    b0d = nc.dram_tensor("B0S", [P, KPC * FT], F32, kind="ExternalInput")
    w1d = nc.dram_tensor("W1S", [P, KPC * FT * FT, P], F32R, kind="ExternalInput")
    b1d = nc.dram_tensor("B1S", [P, KPC * FT], F32, kind="ExternalInput")
    w2d = nc.dram_tensor("W2S", [P, KPC * FT * FT, P], F32R, kind="ExternalInput")
    b2d = nc.dram_tensor("B2S", [P, KPC * FT], F32, kind="ExternalInput")
    w3d = nc.dram_tensor("W3S", [P, KPC * FT], F32, kind="ExternalInput")
    b3d = nc.dram_tensor("B3S", [P, KPC], F32, kind="ExternalInput")  # b3/128
    wbd = nc.dram_tensor("WBS", [P, KPC * 2 * D], F32, kind="ExternalInput")
    outd = nc.dram_tensor("OUT", [2, P, NT], F32, kind="ExternalOutput")

    with tile.TileContext(nc) as tc:
        with (
            tc.tile_pool(name="const", bufs=1) as const,
            tc.tile_pool(name="win", bufs=1) as win,
            tc.tile_pool(name="wtmp", bufs=2) as wtmp,
            tc.tile_pool(name="xin", bufs=3) as xin,
            tc.tile_pool(name="hbuf", bufs=2) as hbuf,
            tc.tile_pool(name="sfold", bufs=2) as spool,
            tc.tile_pool(name="fin", bufs=1) as fin,
            tc.tile_pool(name="psum", bufs=3, space="PSUM") as psum,
            tc.tile_pool(name="psum_s", bufs=2, space="PSUM") as psum_s,
        ):
            # resident constants
            w0 = const.tile([D, KPC * W], F32R)
            nc.sync.dma_start(w0[:], w0d[:])
            b0 = const.tile([P, KPC * FT], F32)
            nc.sync.dma_start(b0[:], b0d[:])
            w1 = const.tile([P, KPC * FT * FT, P], F32R)
            nc.sync.dma_start(w1[:], w1d[:])
            b1 = const.tile([P, KPC * FT], F32)
            nc.sync.dma_start(b1[:], b1d[:])
            w2 = const.tile([P, KPC * FT * FT, P], F32R)
            nc.sync.dma_start(w2[:], w2d[:])
            b2 = const.tile([P, KPC * FT], F32)
            nc.sync.dma_start(b2[:], b2d[:])
            w3 = const.tile([P, KPC * FT], F32)
            nc.sync.dma_start(w3[:], w3d[:])
            b3 = const.tile([P, KPC], F32)
            nc.sync.dma_start(b3[:], b3d[:])
            wb = const.tile([P, KPC * 2 * D], F32)
            nc.sync.dma_start(wb[:], wbd[:])
            xpt = const.tile([P, D * NT], F32)
            nc.sync.dma_start(xpt[:], xPT[:])
            ones_f = const.tile([P, 2], F32)
            nc.vector.memset(ones_f[:], 1.0)
            ones = const.tile([P, 2], F32R)
            nc.vector.tensor_copy(ones[:], ones_f[:])

            loop = tc.For_i(0, repeat, 1) if repeat > 1 else contextlib.nullcontext()
            with loop:
                # cosine partition-of-unity windows, point-partition layout.
                # 0.25*(1-cos(pi a))*(1-cos(pi b)) = (sin(pi a/2)*sin(pi b/2))^2
                # so w_raw = (prod over 4 ramp sides of sin(pi t/2))^2, exactly.
                wvals = win.tile([P, KPC, NT], F32)
                subv = win.tile([P, KPC, NT], F32)
                for s in range(KPC):
                    vs = []
                    for d in range(D):
                        for side in range(2):  # 0 = left ramp, 1 = right ramp
                            col = s * 4 + side * 2 + d
                            u = wtmp.tile([P, NT], F32, tag="w_u")
                            sc = 2.5 if side == 0 else -2.5  # 1/(2*TW)
                            nc.vector.tensor_scalar(
                                u[:], xpt[:, d * NT:(d + 1) * NT],
                                sc, wb[:, col:col + 1], op0=ALU.mult, op1=ALU.add,
                            )
                            nc.vector.tensor_scalar(
                                u[:], u[:], 1.0, 0.0, op0=ALU.min, op1=ALU.max,
                            )
                            v = wtmp.tile([P, NT], F32, tag=f"w_v{side}{d}")
                            nc.scalar.activation(
                                v[:], u[:], AF.Sin, scale=math.pi / 2,
                            )
                            vs.append(v)
                    pa = wtmp.tile([P, NT], F32, tag="w_pa")
                    nc.vector.tensor_tensor(pa[:], vs[0][:], vs[1][:], ALU.mult)
                    pb = wtmp.tile([P, NT], F32, tag="w_pb")
                    nc.vector.tensor_tensor(pb[:], vs[2][:], vs[3][:], ALU.mult)
                    nc.vector.tensor_tensor(pa[:], pa[:], pb[:], ALU.mult)
                    nc.vector.tensor_tensor(wvals[:, s, :], pa[:], pa[:], ALU.mult)

                # main MLP pipeline, chunked over points
                for c in range(NCH):
                    x2c = xin.tile([D, CH], F32R)
                    nc.sync.dma_start(x2c[:], xT[:, c * CH:(c + 1) * CH])
                    for s in range(KPC):
                        # layer 0: [D]->[W], contraction over D=2
                        ps0 = []
                        for mt in range(FT):
                            pt = psum.tile([P, CH], F32, tag="mm")
                            for j in range(CH // SUB):
                                js = slice(j * SUB, (j + 1) * SUB)
                                nc.tensor.matmul(
                                    pt[:, js],
                                    w0[:, (s * FT + mt) * P:(s * FT + mt + 1) * P],
                                    x2c[:, js],
                                    start=True, stop=True,
                                )
                            ps0.append(pt)
                        h0 = hbuf.tile([P, FT, CH], F32R, tag="h0")
                        for mt in range(FT):
                            bi = s * FT + mt
                            nc.scalar.activation(
                                h0[:, mt, :], ps0[mt][:], AF.Tanh,
                                bias=b0[:, bi:bi + 1],
                            )
                        # hidden layers 1 and 2
                        hprev = h0
                        for wl, bl, htag in ((w1, b1, "h1"), (w2, b2, "h2")):
                            psl = []
                            for mt in range(FT):
                                pt = psum.tile([P, CH], F32, tag="mm")
                                for j in range(CH // SUB):
                                    js = slice(j * SUB, (j + 1) * SUB)
                                    for ct in range(FT):
                                        nc.tensor.matmul(
                                            pt[:, js],
                                            wl[:, (s * FT + mt) * FT + ct, :],
                                            hprev[:, ct, js],
                                            start=(ct == 0), stop=(ct == FT - 1),
                                        )
                                psl.append(pt)
                            hn = hbuf.tile([P, FT, CH], F32R, tag=htag)
                            for mt in range(FT):
                                bi = s * FT + mt
                                nc.scalar.activation(
                                    hn[:, mt, :], psl[mt][:], AF.Tanh,
                                    bias=bl[:, bi:bi + 1],
                                )
                            hprev = hn
                        # W3 fold: sf[f,n] = sum_ft h2[ft][f,n]*W3[ft][f] + b3/128
                        sf = spool.tile([P, CH], F32R, tag="sf")
                        nc.vector.tensor_scalar(
                            sf[:], hprev[:, 0, :],
                            w3[:, s * FT:s * FT + 1], b3[:, s:s + 1],
                            op0=ALU.mult, op1=ALU.add,
                        )
                        sf2 = spool.tile([P, CH], F32R, tag="sf2")
                        nc.vector.tensor_scalar(
                            sf2[:], hprev[:, 1, :],
                            w3[:, s * FT + 1:s * FT + 2], None, op0=ALU.mult,
                        )
                        nc.vector.tensor_tensor(sf[:], sf[:], sf2[:], ALU.add)
                        # partition reduction: sf block as the stationary
                        # operand x ones -> psum [128 points, 1] per block,
                        # i.e. sub lands directly in point-partition layout
                        # (n = t*128 + p).
                        # fp32r ISA needs even innermost free counts, so the
                        # reduce matmul emits 2 identical columns per block
                        nblk = CH // P
                        pss = psum_s.tile([P, nblk, 2], F32, tag="ps_s")
                        for nb in range(nblk):
                            nc.tensor.matmul(
                                pss[:, nb, :],
                                sf[:, nb * P:(nb + 1) * P],
                                ones[:],
                                start=True, stop=True,
                            )
                        nc.vector.tensor_copy(
                            subv[:, s, c * nblk:(c + 1) * nblk], pss[:, :, 0],
                        )

                # final combine: num = sum_s sub_s * w_s ; den = sum_s w_s
                num = fin.tile([P, NT], F32)
                den = fin.tile([P, NT], F32)
                for s in range(KPC):
                    prod = fin.tile([P, NT], F32, tag=f"prod{s}")
                    nc.vector.tensor_tensor(
                        prod[:], subv[:, s, :], wvals[:, s, :], ALU.mult,
                    )
                    if s == 0:
                        nc.vector.tensor_copy(num[:], prod[:])
                        nc.vector.tensor_copy(den[:], wvals[:, s, :])
                    else:
                        nc.vector.tensor_tensor(num[:], num[:], prod[:], ALU.add)
                        nc.vector.tensor_tensor(
                            den[:], den[:], wvals[:, s, :], ALU.add,
                        )
                nc.sync.dma_start(outd[0], num[:])
                nc.sync.dma_start(outd[1], den[:])

    nc.compile()
    return nc


_PROGRAM = None


def _program():
    global _PROGRAM
    if _PROGRAM is None:
        _PROGRAM = _build_program()
    return _PROGRAM


def _prep_in_maps(x, W0, b0, W1, b1, W2, b2, W3, b3, xmins, xmaxs):
    f32 = np.float32
    x = np.asarray(x, f32)
    center = (xmins + xmaxs) * 0.5
    scale = np.maximum((xmaxs - xmins) * 0.5, 1e-9).astype(f32)

    xT = np.ascontiguousarray(x.T)  # [D, N]
    # point-partition layout: xpt[p, d*NT + t] = x[p*NT + t, d]
    # t-major point mapping: n = t*P + p  ->  xpt[p, d*NT + t] = x[t*P + p, d]
    xpt = np.ascontiguousarray(
        x.reshape(NT, P, D).transpose(1, 2, 0).reshape(P, D * NT)
    )

    in_maps = []
    for core in range(NCORES):
        ks = [core * KPC + s for s in range(KPC)]
        w0s = np.empty((D, KPC * W), f32)
        b0s = np.empty((P, KPC * FT), f32)
        w1s = np.empty((P, KPC * FT * FT, P), f32)
        b1s = np.empty((P, KPC * FT), f32)
        w2s = np.empty((P, KPC * FT * FT, P), f32)
        b2s = np.empty((P, KPC * FT), f32)
        w3s = np.empty((P, KPC * FT), f32)
        b3s = np.empty((P, KPC), f32)
        wbs = np.empty((P, KPC * 2 * D), f32)
        for s, k in enumerate(ks):
            # fold input normalization into layer 0
            w0eff = (W0[k] / scale[k][:, None]).astype(f32)  # [D, W]
            b0eff = (b0[k] - (center[k] / scale[k]) @ W0[k]).astype(f32)  # [W]
            w0s[:, s * W:(s + 1) * W] = w0eff
            for mt in range(FT):
                b0s[:, s * FT + mt] = b0eff[mt * P:(mt + 1) * P]
                b1s[:, s * FT + mt] = b1[k][mt * P:(mt + 1) * P]
                b2s[:, s * FT + mt] = b2[k][mt * P:(mt + 1) * P]
                w3s[:, s * FT + mt] = W3[k][mt * P:(mt + 1) * P, 0]
                for ct in range(FT):
                    w1s[:, (s * FT + mt) * FT + ct, :] = (
                        W1[k][ct * P:(ct + 1) * P, mt * P:(mt + 1) * P]
                    )
                    w2s[:, (s * FT + mt) * FT + ct, :] = (
                        W2[k][ct * P:(ct + 1) * P, mt * P:(mt + 1) * P]
                    )
            b3s[:, s] = b3[k][0] / P
            for dd in range(D):
                # left: u = x*2.5 + (TW - xmin)/(2 TW)
                wbs[:, s * 4 + 0 * 2 + dd] = (TW - xmins[k, dd]) / (2 * TW)
                # right: u = -x*2.5 + (xmax + TW)/(2 TW)
                wbs[:, s * 4 + 1 * 2 + dd] = (xmaxs[k, dd] + TW) / (2 * TW)
        in_maps.append({
            "XT": xT, "XPT": xpt,
            "W0S": w0s, "B0S": b0s,
            "W1S": w1s, "B1S": b1s,
            "W2S": w2s, "B2S": b2s,
            "W3S": w3s, "B3S": b3s,
            "WBS": wbs,
        })
    return in_maps


def kernel(x, W0, b0, W1, b1, W2, b2, W3, b3, xmins, xmaxs):
    args = [np.asarray(a, np.float32) for a in
            (x, W0, b0, W1, b1, W2, b2, W3, b3, xmins, xmaxs)]
    in_maps = _prep_in_maps(*args)
    nc = _program()
    res = run_bass_kernel_spmd(nc, in_maps, list(range(NCORES)))
    num = np.zeros((P, NT), np.float64)
    den = np.zeros((P, NT), np.float64)
    for i in range(NCORES):
        out = res.results[i]["OUT"]
        num += out[0]
        den += out[1]
    num = num.astype(np.float32)
    den = den.astype(np.float32)
    result = num / (den + np.float32(1e-9))
    # invert t-major mapping: n = t*P + p -> flatten [NT, P]
    return result.T.reshape(N, OUT_DIM).astype(np.float32)


# revision 20
# speedup vs baseline: 1.4500x; 1.4500x over previous
"""FBPINN (16 subdomain MLPs over [0,1]^2, cosine partition-of-unity windows)
as a Trainium2 Bass kernel, expert-parallel over 8 NeuronCores.

Sharding: subdomain axis K=16 split 2-per-core. Each core runs its two
subnets on all N points and emits partial (numerator, denominator) sums of
the window-weighted combine; the host adds partials across cores and divides.
This is exact because the window normalization is a ratio of K-sums.

Host-side preprocessing folds the per-subdomain input normalization
xn = (x - center)/scale into the layer-0 weights/bias, so the device
only computes tanh-MLP layers + windows. Matmuls run in float32r
(single-pass PE mode, ~4x the fp32 rate; ~1e-3 relative rounding).
"""

import contextlib
import math

import numpy as np

import concourse.bacc as bacc
import concourse.mybir as mybir
import concourse.tile as tile
from concourse.bass_utils import run_bass_kernel_spmd

# problem constants (hardcoded per harness contract)
K, D, N, W, OUT_DIM = 16, 2, 16384, 256, 1
TW = 0.2
NCORES = 8
KPC = K // NCORES  # subdomains per core
P = 128
NT = N // P        # 128 columns in point-partition layout
CH = 1024          # point chunk through the MLP pipeline
NCH = N // CH
SUB = 512          # matmul moving-operand subchunk
FT = W // P        # feature tiles per hidden layer (2)

F32 = mybir.dt.float32
F32R = mybir.dt.float32r
AF = mybir.ActivationFunctionType
ALU = mybir.AluOpType


def _build_program(repeat=1):
    nc = bacc.Bacc("TRN2", target_bir_lowering=False, debug=False, num_devices=NCORES)

    xT = nc.dram_tensor("XT", [D, N], F32R, kind="ExternalInput")
    xPT = nc.dram_tensor("XPT", [P, D * NT], F32, kind="ExternalInput")
    w0d = nc.dram_tensor("W0S", [D, KPC * W], F32R, kind="ExternalInput")
    b0d = nc.dram_tensor("B0S", [P, KPC * FT], F32, kind="ExternalInput")
    w1d = nc.dram_tensor("W1S", [P, KPC * FT * FT, P], F32R, kind="ExternalInput")
    b1d = nc.dram_tensor("B1S", [P, KPC * FT], F32, kind="ExternalInput")
    w2d = nc.dram_tensor("W2S", [P, KPC * FT * FT, P], F32R, kind="ExternalInput")
    b2d = nc.dram_tensor("B2S", [P, KPC * FT], F32, kind="ExternalInput")
    w3d = nc.dram_tensor("W3S", [P, KPC * FT], F32, kind="ExternalInput")
    b3d = nc.dram_tensor("B3S", [P, KPC], F32, kind="ExternalInput")  # b3/128
    wbd = nc.dram_tensor("WBS", [P, KPC * 2 * D], F32, kind="ExternalInput")
    outd = nc.dram_tensor("OUT", [2, P, NT], F32, kind="ExternalOutput")

    scratch = nc.dram_tensor("SUBSCRATCH", [KPC, 1, N], F32)

    with tile.TileContext(nc) as tc:
        with (
            tc.tile_pool(name="const", bufs=1) as const,
            tc.tile_pool(name="win", bufs=1) as win,
            tc.tile_pool(name="wtmp", bufs=2) as wtmp,
            tc.tile_pool(name="xin", bufs=3) as xin,
            tc.tile_pool(name="hbuf", bufs=2) as hbuf,
            tc.tile_pool(name="sfold", bufs=2) as spool,
            tc.tile_pool(name="stage", bufs=3) as stage,
            tc.tile_pool(name="fin", bufs=1) as fin,
            tc.tile_pool(name="psum", bufs=3, space="PSUM") as psum,
            tc.tile_pool(name="psum_s", bufs=2, space="PSUM") as psum_s,
        ):
            # resident constants
            w0 = const.tile([D, KPC * W], F32R)
            nc.sync.dma_start(w0[:], w0d[:])
            b0 = const.tile([P, KPC * FT], F32)
            nc.sync.dma_start(b0[:], b0d[:])
            w1 = const.tile([P, KPC * FT * FT, P], F32R)
            nc.sync.dma_start(w1[:], w1d[:])
            b1 = const.tile([P, KPC * FT], F32)
            nc.sync.dma_start(b1[:], b1d[:])
            w2 = const.tile([P, KPC * FT * FT, P], F32R)
            nc.sync.dma_start(w2[:], w2d[:])
            b2 = const.tile([P, KPC * FT], F32)
            nc.sync.dma_start(b2[:], b2d[:])
            w3 = const.tile([P, KPC * FT], F32)
            nc.sync.dma_start(w3[:], w3d[:])
            b3 = const.tile([P, KPC], F32)
            nc.sync.dma_start(b3[:], b3d[:])
            wb = const.tile([P, KPC * 2 * D], F32)
            nc.sync.dma_start(wb[:], wbd[:])
            xpt = const.tile([P, D * NT], F32)
            nc.sync.dma_start(xpt[:], xPT[:])
            ones_f = const.tile([P, 1], F32)
            nc.vector.memset(ones_f[:], 1.0)
            ones = const.tile([P, 1], F32R)
            nc.vector.tensor_copy(ones[:], ones_f[:])

            loop = tc.For_i(0, repeat, 1) if repeat > 1 else contextlib.nullcontext()
            with loop:
                # cosine partition-of-unity windows, point-partition layout.
                # 0.25*(1-cos(pi a))*(1-cos(pi b)) = (sin(pi a/2)*sin(pi b/2))^2
                # so w_raw = (prod over 4 ramp sides of sin(pi t/2))^2, exactly.
                wvals = win.tile([P, KPC, NT], F32)
                for s in range(KPC):
                    vs = []
                    for d in range(D):
                        for side in range(2):  # 0 = left ramp, 1 = right ramp
                            col = s * 4 + side * 2 + d
                            u = wtmp.tile([P, NT], F32, tag="w_u")
                            sc = 2.5 if side == 0 else -2.5  # 1/(2*TW)
                            nc.vector.tensor_scalar(
                                u[:], xpt[:, d * NT:(d + 1) * NT],
                                sc, wb[:, col:col + 1], op0=ALU.mult, op1=ALU.add,
                            )
                            nc.vector.tensor_scalar(
                                u[:], u[:], 1.0, 0.0, op0=ALU.min, op1=ALU.max,
                            )
                            v = wtmp.tile([P, NT], F32, tag=f"w_v{side}{d}")
                            nc.scalar.activation(
                                v[:], u[:], AF.Sin, scale=math.pi / 2,
                            )
                            vs.append(v)
                    pa = wtmp.tile([P, NT], F32, tag="w_pa")
                    nc.vector.tensor_tensor(pa[:], vs[0][:], vs[1][:], ALU.mult)
                    pb = wtmp.tile([P, NT], F32, tag="w_pb")
                    nc.vector.tensor_tensor(pb[:], vs[2][:], vs[3][:], ALU.mult)
                    nc.vector.tensor_tensor(pa[:], pa[:], pb[:], ALU.mult)
                    nc.vector.tensor_tensor(wvals[:, s, :], pa[:], pa[:], ALU.mult)

                # main MLP pipeline, chunked over points; the two
                # subdomain streams are interleaved stage-by-stage so the PE
                # works on stream B's matmuls while ACT drains stream A.
                for c in range(NCH):
                    x2c = xin.tile([D, CH], F32R)
                    nc.sync.dma_start(x2c[:], xT[:, c * CH:(c + 1) * CH])
                    # layer 0, both streams
                    ps0 = {}
                    for s in range(KPC):
                        for mt in range(FT):
                            pt = psum.tile([P, CH], F32, tag="mm")
                            for j in range(CH // SUB):
                                js = slice(j * SUB, (j + 1) * SUB)
                                nc.tensor.matmul(
                                    pt[:, js],
                                    w0[:, (s * FT + mt) * P:(s * FT + mt + 1) * P],
                                    x2c[:, js],
                                    start=True, stop=True,
                                )
                            ps0[s, mt] = pt
                    hcur = {}
                    for s in range(KPC):
                        h0 = hbuf.tile([P, FT, CH], F32R, tag=f"h0_{s}")
                        for mt in range(FT):
                            nc.scalar.activation(
                                h0[:, mt, :], ps0[s, mt][:], AF.Tanh,
                                bias=b0[:, s * FT + mt:s * FT + mt + 1],
                            )
                        hcur[s] = h0
                    # hidden layers 1 and 2, both streams per stage
                    for wl, bl, htag in ((w1, b1, "h1"), (w2, b2, "h2")):
                        psl = {}
                        for s in range(KPC):
                            for mt in range(FT):
                                pt = psum.tile([P, CH], F32, tag="mm")
                                for j in range(CH // SUB):
                                    js = slice(j * SUB, (j + 1) * SUB)
                                    for ct in range(FT):
                                        nc.tensor.matmul(
                                            pt[:, js],
                                            wl[:, (s * FT + mt) * FT + ct, :],
                                            hcur[s][:, ct, js],
                                            start=(ct == 0), stop=(ct == FT - 1),
                                        )
                                psl[s, mt] = pt
                        hnxt = {}
                        for s in range(KPC):
                            hn = hbuf.tile([P, FT, CH], F32R, tag=f"{htag}_{s}")
                            for mt in range(FT):
                                nc.scalar.activation(
                                    hn[:, mt, :], psl[s, mt][:], AF.Tanh,
                                    bias=bl[:, s * FT + mt:s * FT + mt + 1],
                                )
                            hnxt[s] = hn
                        hcur = hnxt
                    # W3 fold + partition reduction via ones-matmul
                    for s in range(KPC):
                        sf = spool.tile([P, CH], F32R, tag=f"sf_{s}")
                        nc.vector.tensor_scalar(
                            sf[:], hcur[s][:, 0, :],
                            w3[:, s * FT:s * FT + 1], b3[:, s:s + 1],
                            op0=ALU.mult, op1=ALU.add,
                        )
                        sf2 = spool.tile([P, CH], F32R, tag=f"sf2_{s}")
                        nc.vector.tensor_scalar(
                            sf2[:], hcur[s][:, 1, :],
                            w3[:, s * FT + 1:s * FT + 2], None, op0=ALU.mult,
                        )
                        nc.vector.tensor_tensor(sf[:], sf[:], sf2[:], ALU.add)
                        for j in range(CH // SUB):
                            js = slice(j * SUB, (j + 1) * SUB)
                            pss = psum_s.tile([1, SUB], F32, tag="ps_s")
                            nc.tensor.matmul(
                                pss[:], ones[:], sf[:, js], start=True, stop=True,
                            )
                            row = stage.tile([1, SUB], F32, tag="row")
                            nc.vector.tensor_copy(row[:], pss[:])
                            off = c * CH + j * SUB
                            nc.sync.dma_start(
                                scratch[s, :, off:off + SUB], row[:],
                            )

                # final combine: num = sum_s sub_s * w_s ; den = sum_s w_s
                num = fin.tile([P, NT], F32)
                den = fin.tile([P, NT], F32)
                for s in range(KPC):
                    subf = fin.tile([P, NT], F32, tag=f"subf{s}")
                    nc.sync.dma_start(
                        subf[:], scratch[s, 0, :].rearrange("(p t) -> p t", p=P),
                    )
                    prod = fin.tile([P, NT], F32, tag=f"prod{s}")
                    nc.vector.tensor_tensor(
                        prod[:], subf[:], wvals[:, s, :], ALU.mult,
                    )
                    if s == 0:
                        nc.vector.tensor_copy(num[:], prod[:])
                        nc.vector.tensor_copy(den[:], wvals[:, s, :])
                    else:
                        nc.vector.tensor_tensor(num[:], num[:], prod[:], ALU.add)
                        nc.vector.tensor_tensor(
                            den[:], den[:], wvals[:, s, :], ALU.add,
                        )
                nc.sync.dma_start(outd[0], num[:])
                nc.sync.dma_start(outd[1], den[:])

    nc.compile()
    return nc


_PROGRAM = None


def _program():
    global _PROGRAM
    if _PROGRAM is None:
        _PROGRAM = _build_program()
    return _PROGRAM


def _prep_in_maps(x, W0, b0, W1, b1, W2, b2, W3, b3, xmins, xmaxs):
    f32 = np.float32
    x = np.asarray(x, f32)
    center = (xmins + xmaxs) * 0.5
    scale = np.maximum((xmaxs - xmins) * 0.5, 1e-9).astype(f32)

    xT = np.ascontiguousarray(x.T)  # [D, N]
    # point-partition layout: xpt[p, d*NT + t] = x[p*NT + t, d]
    # p-major point mapping: n = p*NT + t  ->  xpt[p, d*NT + t] = x[p*NT + t, d]
    xpt = np.ascontiguousarray(
        x.reshape(P, NT, D).transpose(0, 2, 1).reshape(P, D * NT)
    )

    in_maps = []
    for core in range(NCORES):
        ks = [core * KPC + s for s in range(KPC)]
        w0s = np.empty((D, KPC * W), f32)
        b0s = np.empty((P, KPC * FT), f32)
        w1s = np.empty((P, KPC * FT * FT, P), f32)
        b1s = np.empty((P, KPC * FT), f32)
        w2s = np.empty((P, KPC * FT * FT, P), f32)
        b2s = np.empty((P, KPC * FT), f32)
        w3s = np.empty((P, KPC * FT), f32)
        b3s = np.empty((P, KPC), f32)
        wbs = np.empty((P, KPC * 2 * D), f32)
        for s, k in enumerate(ks):
            # fold input normalization into layer 0
            w0eff = (W0[k] / scale[k][:, None]).astype(f32)  # [D, W]
            b0eff = (b0[k] - (center[k] / scale[k]) @ W0[k]).astype(f32)  # [W]
            w0s[:, s * W:(s + 1) * W] = w0eff
            for mt in range(FT):
                b0s[:, s * FT + mt] = b0eff[mt * P:(mt + 1) * P]
                b1s[:, s * FT + mt] = b1[k][mt * P:(mt + 1) * P]
                b2s[:, s * FT + mt] = b2[k][mt * P:(mt + 1) * P]
                w3s[:, s * FT + mt] = W3[k][mt * P:(mt + 1) * P, 0]
                for ct in range(FT):
                    w1s[:, (s * FT + mt) * FT + ct, :] = (
                        W1[k][ct * P:(ct + 1) * P, mt * P:(mt + 1) * P]
                    )
                    w2s[:, (s * FT + mt) * FT + ct, :] = (
                        W2[k][ct * P:(ct + 1) * P, mt * P:(mt + 1) * P]
                    )
            b3s[:, s] = b3[k][0] / P
            for dd in range(D):
                # left: u = x*2.5 + (TW - xmin)/(2 TW)
                wbs[:, s * 4 + 0 * 2 + dd] = (TW - xmins[k, dd]) / (2 * TW)
                # right: u = -x*2.5 + (xmax + TW)/(2 TW)
                wbs[:, s * 4 + 1 * 2 + dd] = (xmaxs[k, dd] + TW) / (2 * TW)
        in_maps.append({
            "XT": xT, "XPT": xpt,
            "W0S": w0s, "B0S": b0s,
            "W1S": w1s, "B1S": b1s,
            "W2S": w2s, "B2S": b2s,
            "W3S": w3s, "B3S": b3s,
            "WBS": wbs,
        })
    return in_maps


def kernel(x, W0, b0, W1, b1, W2, b2, W3, b3, xmins, xmaxs):
    args = [np.asarray(a, np.float32) for a in
            (x, W0, b0, W1, b1, W2, b2, W3, b3, xmins, xmaxs)]
    in_maps = _prep_in_maps(*args)
    nc = _program()
    res = run_bass_kernel_spmd(nc, in_maps, list(range(NCORES)))
    num = np.zeros((P, NT), np.float64)
    den = np.zeros((P, NT), np.float64)
    for i in range(NCORES):
        out = res.results[i]["OUT"]
        num += out[0]
        den += out[1]
    num = num.astype(np.float32)
    den = den.astype(np.float32)
    result = num / (den + np.float32(1e-9))
    # invert p-major mapping: n = p*NT + t
    return result.reshape(N, OUT_DIM).astype(np.float32)


# revision 21
# speedup vs baseline: 1.5088x; 1.0406x over previous
"""FBPINN (16 subdomain MLPs over [0,1]^2, cosine partition-of-unity windows)
as a Trainium2 Bass kernel, expert-parallel over 8 NeuronCores.

Sharding: subdomain axis K=16 split 2-per-core. Each core runs its two
subnets on all N points and emits partial (numerator, denominator) sums of
the window-weighted combine; the host adds partials across cores and divides.
This is exact because the window normalization is a ratio of K-sums.

Host-side preprocessing folds the per-subdomain input normalization
xn = (x - center)/scale into the layer-0 weights/bias, so the device
only computes tanh-MLP layers + windows. Matmuls run in float32r
(single-pass PE mode, ~4x the fp32 rate; ~1e-3 relative rounding).
"""

import contextlib
import math

import numpy as np

import concourse.bacc as bacc
import concourse.mybir as mybir
import concourse.tile as tile
from concourse.bass_utils import run_bass_kernel_spmd

# problem constants (hardcoded per harness contract)
K, D, N, W, OUT_DIM = 16, 2, 16384, 256, 1
TW = 0.2
NCORES = 8
KPC = K // NCORES  # subdomains per core
P = 128
NT = N // P        # 128 columns in point-partition layout
CH = 1024          # point chunk through the MLP pipeline
NCH = N // CH
SUB = 512          # matmul moving-operand subchunk
FT = W // P        # feature tiles per hidden layer (2)

F32 = mybir.dt.float32
F32R = mybir.dt.float32r
AF = mybir.ActivationFunctionType
ALU = mybir.AluOpType


def _build_program(repeat=1):
    nc = bacc.Bacc("TRN2", target_bir_lowering=False, debug=False, num_devices=NCORES)

    xT = nc.dram_tensor("XT", [D, N], F32R, kind="ExternalInput")
    xPT = nc.dram_tensor("XPT", [P, D * NT], F32, kind="ExternalInput")
    w0d = nc.dram_tensor("W0S", [D, KPC * W], F32R, kind="ExternalInput")
    b0d = nc.dram_tensor("B0S", [P, KPC * FT], F32, kind="ExternalInput")
    w1d = nc.dram_tensor("W1S", [P, KPC * FT * FT, P], F32R, kind="ExternalInput")
    b1d = nc.dram_tensor("B1S", [P, KPC * FT], F32, kind="ExternalInput")
    w2d = nc.dram_tensor("W2S", [P, KPC * FT * FT, P], F32R, kind="ExternalInput")
    b2d = nc.dram_tensor("B2S", [P, KPC * FT], F32, kind="ExternalInput")
    w3d = nc.dram_tensor("W3S", [P, KPC * FT], F32, kind="ExternalInput")
    b3d = nc.dram_tensor("B3S", [P, KPC], F32, kind="ExternalInput")  # b3/128
    wbd = nc.dram_tensor("WBS", [P, KPC * 2 * D], F32, kind="ExternalInput")
    outd = nc.dram_tensor("OUT", [2, P, NT], F32, kind="ExternalOutput")

    scratch = nc.dram_tensor("SUBSCRATCH", [KPC, 1, N], F32)

    with tile.TileContext(nc) as tc:
        with (
            tc.tile_pool(name="const", bufs=1) as const,
            tc.tile_pool(name="win", bufs=1) as win,
            tc.tile_pool(name="wtmp", bufs=2) as wtmp,
            tc.tile_pool(name="xin", bufs=3) as xin,
            tc.tile_pool(name="hbuf", bufs=2) as hbuf,
            tc.tile_pool(name="sfold", bufs=2) as spool,
            tc.tile_pool(name="stage", bufs=3) as stage,
            tc.tile_pool(name="fin", bufs=1) as fin,
            tc.tile_pool(name="psum", bufs=3, space="PSUM") as psum,
            tc.tile_pool(name="psum_s", bufs=2, space="PSUM") as psum_s,
        ):
            # resident constants
            w0 = const.tile([D, KPC * W], F32R)
            nc.sync.dma_start(w0[:], w0d[:])
            b0 = const.tile([P, KPC * FT], F32)
            nc.sync.dma_start(b0[:], b0d[:])
            w1 = const.tile([P, KPC * FT * FT, P], F32R)
            nc.sync.dma_start(w1[:], w1d[:])
            b1 = const.tile([P, KPC * FT], F32)
            nc.sync.dma_start(b1[:], b1d[:])
            w2 = const.tile([P, KPC * FT * FT, P], F32R)
            nc.sync.dma_start(w2[:], w2d[:])
            b2 = const.tile([P, KPC * FT], F32)
            nc.sync.dma_start(b2[:], b2d[:])
            w3 = const.tile([P, KPC * FT], F32)
            nc.sync.dma_start(w3[:], w3d[:])
            b3 = const.tile([P, KPC], F32)
            nc.sync.dma_start(b3[:], b3d[:])
            wb = const.tile([P, KPC * 2 * D], F32)
            nc.sync.dma_start(wb[:], wbd[:])
            xpt = const.tile([P, D * NT], F32)
            nc.sync.dma_start(xpt[:], xPT[:])
            ones_f = const.tile([P, 1], F32)
            nc.vector.memset(ones_f[:], 1.0)
            ones = const.tile([P, 1], F32R)
            nc.vector.tensor_copy(ones[:], ones_f[:])

            loop = tc.For_i(0, repeat, 1) if repeat > 1 else contextlib.nullcontext()
            with loop:
                # main MLP pipeline, chunked over points; the two
                # subdomain streams are interleaved stage-by-stage so the PE
                # works on stream B's matmuls while ACT drains stream A.
                for c in range(NCH):
                    x2c = xin.tile([D, CH], F32R)
                    nc.sync.dma_start(x2c[:], xT[:, c * CH:(c + 1) * CH])
                    # layer 0, both streams
                    ps0 = {}
                    for s in range(KPC):
                        for mt in range(FT):
                            pt = psum.tile([P, CH], F32, tag="mm")
                            for j in range(CH // SUB):
                                js = slice(j * SUB, (j + 1) * SUB)
                                nc.tensor.matmul(
                                    pt[:, js],
                                    w0[:, (s * FT + mt) * P:(s * FT + mt + 1) * P],
                                    x2c[:, js],
                                    start=True, stop=True,
                                )
                            ps0[s, mt] = pt
                    hcur = {}
                    for s in range(KPC):
                        h0 = hbuf.tile([P, FT, CH], F32R, tag=f"h0_{s}")
                        for mt in range(FT):
                            nc.scalar.activation(
                                h0[:, mt, :], ps0[s, mt][:], AF.Tanh,
                                bias=b0[:, s * FT + mt:s * FT + mt + 1],
                            )
                        hcur[s] = h0
                    # hidden layers 1 and 2, both streams per stage
                    for wl, bl, htag in ((w1, b1, "h1"), (w2, b2, "h2")):
                        psl = {}
                        for s in range(KPC):
                            for mt in range(FT):
                                pt = psum.tile([P, CH], F32, tag="mm")
                                for j in range(CH // SUB):
                                    js = slice(j * SUB, (j + 1) * SUB)
                                    for ct in range(FT):
                                        nc.tensor.matmul(
                                            pt[:, js],
                                            wl[:, (s * FT + mt) * FT + ct, :],
                                            hcur[s][:, ct, js],
                                            start=(ct == 0), stop=(ct == FT - 1),
                                        )
                                psl[s, mt] = pt
                        hnxt = {}
                        for s in range(KPC):
                            hn = hbuf.tile([P, FT, CH], F32R, tag=f"{htag}_{s}")
                            for mt in range(FT):
                                nc.scalar.activation(
                                    hn[:, mt, :], psl[s, mt][:], AF.Tanh,
                                    bias=bl[:, s * FT + mt:s * FT + mt + 1],
                                )
                            hnxt[s] = hn
                        hcur = hnxt
                    # W3 fold + partition reduction via ones-matmul
                    for s in range(KPC):
                        sf = spool.tile([P, CH], F32R, tag=f"sf_{s}")
                        nc.vector.tensor_scalar(
                            sf[:], hcur[s][:, 0, :],
                            w3[:, s * FT:s * FT + 1], b3[:, s:s + 1],
                            op0=ALU.mult, op1=ALU.add,
                        )
                        sf2 = spool.tile([P, CH], F32R, tag=f"sf2_{s}")
                        nc.vector.tensor_scalar(
                            sf2[:], hcur[s][:, 1, :],
                            w3[:, s * FT + 1:s * FT + 2], None, op0=ALU.mult,
                        )
                        nc.vector.tensor_tensor(sf[:], sf[:], sf2[:], ALU.add)
                        for j in range(CH // SUB):
                            js = slice(j * SUB, (j + 1) * SUB)
                            pss = psum_s.tile([1, SUB], F32, tag="ps_s")
                            nc.tensor.matmul(
                                pss[:], ones[:], sf[:, js], start=True, stop=True,
                            )
                            row = stage.tile([1, SUB], F32, tag="row")
                            nc.vector.tensor_copy(row[:], pss[:])
                            off = c * CH + j * SUB
                            nc.sync.dma_start(
                                scratch[s, :, off:off + SUB], row[:],
                            )

                # cosine partition-of-unity windows, point-partition layout.
                # 0.25*(1-cos(pi a))*(1-cos(pi b)) = (sin(pi a/2)*sin(pi b/2))^2
                # so w_raw = (prod over 4 ramp sides of sin(pi t/2))^2, exactly.
                wvals = win.tile([P, KPC, NT], F32)
                for s in range(KPC):
                    vs = []
                    for d in range(D):
                        for side in range(2):  # 0 = left ramp, 1 = right ramp
                            col = s * 4 + side * 2 + d
                            u = wtmp.tile([P, NT], F32, tag="w_u")
                            sc = 2.5 if side == 0 else -2.5  # 1/(2*TW)
                            nc.vector.tensor_scalar(
                                u[:], xpt[:, d * NT:(d + 1) * NT],
                                sc, wb[:, col:col + 1], op0=ALU.mult, op1=ALU.add,
                            )
                            nc.vector.tensor_scalar(
                                u[:], u[:], 1.0, 0.0, op0=ALU.min, op1=ALU.max,
                            )
                            v = wtmp.tile([P, NT], F32, tag=f"w_v{side}{d}")
                            nc.scalar.activation(
                                v[:], u[:], AF.Sin, scale=math.pi / 2,
                            )
                            vs.append(v)
                    pa = wtmp.tile([P, NT], F32, tag="w_pa")
                    nc.vector.tensor_tensor(pa[:], vs[0][:], vs[1][:], ALU.mult)
                    pb = wtmp.tile([P, NT], F32, tag="w_pb")
                    nc.vector.tensor_tensor(pb[:], vs[2][:], vs[3][:], ALU.mult)
                    nc.vector.tensor_tensor(pa[:], pa[:], pb[:], ALU.mult)
                    nc.vector.tensor_tensor(wvals[:, s, :], pa[:], pa[:], ALU.mult)

                # final combine: num = sum_s sub_s * w_s ; den = sum_s w_s
                num = fin.tile([P, NT], F32)
                den = fin.tile([P, NT], F32)
                for s in range(KPC):
                    subf = fin.tile([P, NT], F32, tag=f"subf{s}")
                    nc.sync.dma_start(
                        subf[:], scratch[s, 0, :].rearrange("(p t) -> p t", p=P),
                    )
                    prod = fin.tile([P, NT], F32, tag=f"prod{s}")
                    nc.vector.tensor_tensor(
                        prod[:], subf[:], wvals[:, s, :], ALU.mult,
                    )
                    if s == 0:
                        nc.vector.tensor_copy(num[:], prod[:])
                        nc.vector.tensor_copy(den[:], wvals[:, s, :])
                    else:
                        nc.vector.tensor_tensor(num[:], num[:], prod[:], ALU.add)
                        nc.vector.tensor_tensor(
                            den[:], den[:], wvals[:, s, :], ALU.add,
                        )
                nc.sync.dma_start(outd[0], num[:])
                nc.sync.dma_start(outd[1], den[:])

    nc.compile()
    return nc


_PROGRAM = None


def _program():
    global _PROGRAM
    if _PROGRAM is None:
        _PROGRAM = _build_program()
    return _PROGRAM


def _prep_in_maps(x, W0, b0, W1, b1, W2, b2, W3, b3, xmins, xmaxs):
    f32 = np.float32
    x = np.asarray(x, f32)
    center = (xmins + xmaxs) * 0.5
    scale = np.maximum((xmaxs - xmins) * 0.5, 1e-9).astype(f32)

    xT = np.ascontiguousarray(x.T)  # [D, N]
    # point-partition layout: xpt[p, d*NT + t] = x[p*NT + t, d]
    # p-major point mapping: n = p*NT + t  ->  xpt[p, d*NT + t] = x[p*NT + t, d]
    xpt = np.ascontiguousarray(
        x.reshape(P, NT, D).transpose(0, 2, 1).reshape(P, D * NT)
    )

    in_maps = []
    for core in range(NCORES):
        ks = [core * KPC + s for s in range(KPC)]
        w0s = np.empty((D, KPC * W), f32)
        b0s = np.empty((P, KPC * FT), f32)
        w1s = np.empty((P, KPC * FT * FT, P), f32)
        b1s = np.empty((P, KPC * FT), f32)
        w2s = np.empty((P, KPC * FT * FT, P), f32)
        b2s = np.empty((P, KPC * FT), f32)
        w3s = np.empty((P, KPC * FT), f32)
        b3s = np.empty((P, KPC), f32)
        wbs = np.empty((P, KPC * 2 * D), f32)
        for s, k in enumerate(ks):
            # fold input normalization into layer 0
            w0eff = (W0[k] / scale[k][:, None]).astype(f32)  # [D, W]
            b0eff = (b0[k] - (center[k] / scale[k]) @ W0[k]).astype(f32)  # [W]
            w0s[:, s * W:(s + 1) * W] = w0eff
            for mt in range(FT):
                b0s[:, s * FT + mt] = b0eff[mt * P:(mt + 1) * P]
                b1s[:, s * FT + mt] = b1[k][mt * P:(mt + 1) * P]
                b2s[:, s * FT + mt] = b2[k][mt * P:(mt + 1) * P]
                w3s[:, s * FT + mt] = W3[k][mt * P:(mt + 1) * P, 0]
                for ct in range(FT):
                    w1s[:, (s * FT + mt) * FT + ct, :] = (
                        W1[k][ct * P:(ct + 1) * P, mt * P:(mt + 1) * P]
                    )
                    w2s[:, (s * FT + mt) * FT + ct, :] = (
                        W2[k][ct * P:(ct + 1) * P, mt * P:(mt + 1) * P]
                    )
            b3s[:, s] = b3[k][0] / P
            for dd in range(D):
                # left: u = x*2.5 + (TW - xmin)/(2 TW)
                wbs[:, s * 4 + 0 * 2 + dd] = (TW - xmins[k, dd]) / (2 * TW)
                # right: u = -x*2.5 + (xmax + TW)/(2 TW)
                wbs[:, s * 4 + 1 * 2 + dd] = (xmaxs[k, dd] + TW) / (2 * TW)
        in_maps.append({
            "XT": xT, "XPT": xpt,
            "W0S": w0s, "B0S": b0s,
            "W1S": w1s, "B1S": b1s,
            "W2S": w2s, "B2S": b2s,
            "W3S": w3s, "B3S": b3s,
            "WBS": wbs,
        })
    return in_maps


def kernel(x, W0, b0, W1, b1, W2, b2, W3, b3, xmins, xmaxs):
    args = [np.asarray(a, np.float32) for a in
            (x, W0, b0, W1, b1, W2, b2, W3, b3, xmins, xmaxs)]
    in_maps = _prep_in_maps(*args)
    nc = _program()
    res = run_bass_kernel_spmd(nc, in_maps, list(range(NCORES)))
    num = np.zeros((P, NT), np.float64)
    den = np.zeros((P, NT), np.float64)
    for i in range(NCORES):
        out = res.results[i]["OUT"]
        num += out[0]
        den += out[1]
    num = num.astype(np.float32)
    den = den.astype(np.float32)
    result = num / (den + np.float32(1e-9))
    # invert p-major mapping: n = p*NT + t
    return result.reshape(N, OUT_DIM).astype(np.float32)
